# revision 29
# baseline (speedup 1.0000x reference)
"""ECE loss kernel for Trainium2 (Bass/Tile), data-parallel over 8 NeuronCores.

Math (per sample row of logits[N, C]):
  conf = max softmax(x) = max(E) / sum(E),  E = exp(x)
  acc  = (argmax(x) == label)  via  exp(g) == max(E), g = x[i, label_i]
  ece  = sum_b |conf_sum[b] - acc_sum[b]| / N   over 15 real bins

Per-core device work (125k rows as [125 partitions x 1000 samples x 100 cls]),
balanced across ALL engines (the previous version put everything on DVE):
  - DMA   (sync HWDGE only): 13 tiles, up to 5 MB each
  - ACT   : E = exp(x) in place; later all per-bin statistics via
            activation(Relu/Sign, bias=-C, accum_out=...) which gives a free
            per-partition sum of the activated values
  - DVE   : rowmax(E); rowsum for the small lead tiles; final rowsum over 25
            for the big tiles; recip/eq/mul/stt epilogue per chunk
  - GpSimd: two pairwise-ADD tree levels (100->50->25) in place on each big
            tile, via tensor_tensor(add) -- runs after DVE's rowmax read
            (Pool TT supports add/mult but not max)

Per-bin statistics (accumulated per chunk of samples so they overlap the
main loop instead of forming a serial tail):
  wt'(Cb) = sum relu(conf - Cb)        (ACT, 15 ops: Cb in {0} + C_0..C_13)
  nle_b   = sum (conf <= C_b)          (DVE tensor_scalar accum, 14 ops)
  q'(Tb)  = sum sign(v' - Tb)          (ACT, 15 ops) where
            v' = 2*sign(eg - maxE) - conf  (acc1: -conf; acc0: -2-conf),
            Tb = -C_b for b=0..13 and -1.0 for the total-acc count
Host recovers:
  T = wt'(0);  S_b = T - wt'_b - C_b*(N - nle_b)
  A_b = (N + q'_b)/2   (cumulative acc counts);  diffs give per-bin sums.
C_b is the exact f32 boundary: the largest f32 y with f32(15*y) <= b+1, so
binning matches the reference's ceil(conf*15) up to ~1-sample tie effects
(~1e-6 relative on the final ECE).
"""

import os

import numpy as np

import concourse.bass as bass
import concourse.mybir as mybir
import concourse.tile as tile
from concourse.bass_utils import run_bass_kernel_spmd

F32 = mybir.dt.float32
F16 = mybir.dt.float16
ALU = mybir.AluOpType
AX = mybir.AxisListType
ACTF = mybir.ActivationFunctionType

N = 1_000_000
C = 100
NCORES = 8
ROWS = N // NCORES          # 125000 rows per core
P = 125                     # SBUF partitions used
SPP = ROWS // P             # 1000 samples per partition

# small tiles at both ends: fast pipeline ramp-up AND a short serial tail
SIZES = [12, 13, 25, 50, 100, 100, 100, 100,   # chunk 0 (500)
         100, 100, 100,                        # chunk 1 (300)
         100, 50,                              # chunk 2 (150)
         25, 13, 12]                           # chunk 3 (50)
CHUNKS = [(0, 500), (500, 800), (800, 950), (950, 1000)]
CHUNK_LAST_TILE = [7, 10, 12, 15]
NCHUNK = 4
ACT_CHUNKS = (0, 1)        # big chunks bin on ACT (relu/sign accum) mid
                           # loop; the small tail chunks bin on DVE with
                           # direct counts (tiny ops, ~5ns/sample)
DVE_FULL_K = 25            # tiles with k <= this do the row sum on DVE too
NSLOT = 44

LAST_RESULTS = None         # stashed BassKernelResults for test harness


def _bin_thresholds():
    """C_b = largest f32 y such that f32(15*y) <= b+1, for b = 0..14."""
    thr = []
    for b in range(15):
        tgt = np.float32(b + 1)

        def f(v):
            return np.float32(np.float32(15.0) * v)

        y = np.float32((b + 1) / 15.0)
        if f(y) <= tgt:
            while True:
                y2 = np.nextafter(y, np.float32(np.inf))
                if f(y2) <= tgt:
                    y = y2
                else:
                    break
        else:
            while f(y) > tgt:
                y = np.nextafter(y, np.float32(-np.inf))
        thr.append(np.float32(y))
    return thr


THR = _bin_thresholds()                       # 15 values, b = 0..14

# bias constants shipped as a tiny input tensor (the const-AP pool only has
# 0.0/1.0 pre-registered):  [0] = 0.0 (wt base),  [1+b] = -C_b (wt relu),
# [15+b] = +C_b (av sign on v' = 2*sign(d) - conf),  [29] = +1.0 (acc count)
NCONST = 30
CVEC = np.zeros(NCONST, np.float32)
for _b in range(14):
    CVEC[1 + _b] = -THR[_b]
    CVEC[15 + _b] = THR[_b]
CVEC[29] = np.float32(1.0)


def _fix_sync(nc):
    """Instruction encodings only carry 2 sync-command slots (completion
    update takes one), so every instruction should hold <= 1 wait.  Tile's
    sem emission is not transitively minimal, so: (1) drop waits implied
    transitively through other waits / same-engine program order; (2) split
    any leftover multi-wait instruction into a chain of presync drains."""
    import bisect
    import re

    import bass_rust as _br

    TICK = re.compile(r"^(Activation|DVE|PE|Pool|SP|DMAHW\d+|DMASW\d+)_\d+$")
    ASYNC_T = {"InstDMACopy", "InstTriggerDma"}

    insts = []
    for bb in nc.m.functions[0].blocks:
        for ins in bb.instructions:
            insts.append(ins)
    n = len(insts)

    # producer map: tick sem -> sorted cumulative values + producing inst idx
    prod_vals, prod_idx = {}, {}
    own_updates = [[] for _ in range(n)]
    cum = {}
    for idx, ins in enumerate(insts):
        si = ins.sync_info
        if si is None:
            continue
        for u in si.on_update:
            nm = u.ant_name
            if not nm or not TICK.match(nm):
                continue
            if u.update_mode not in ("sem-inc", "sem-add-imm"):
                continue
            v = cum.get(nm, 0) + (u.update_value or 1)
            cum[nm] = v
            prod_vals.setdefault(nm, []).append(v)
            prod_idx.setdefault(nm, []).append(idx)
            own_updates[idx].append((nm, v))

    def producer(nm, val):
        vs = prod_vals.get(nm)
        if not vs:
            return None
        k = bisect.bisect_left(vs, val)
        if k >= len(vs):
            return None
        return prod_idx[nm][k]

    prev_idx = [None] * n
    last = {}
    for idx, ins in enumerate(insts):
        e = str(getattr(ins, "engine", None))
        prev_idx[idx] = last.get(e)
        last[e] = idx

    # before[i]: sem clock guaranteed when inst i issues (incl its waits)
    # after[i]: clock guaranteed when inst i COMPLETES (incl own updates)
    before = [None] * n
    after = [None] * n

    def wait_producers(i):
        si = insts[i].sync_info
        out = []
        for w in (si.on_wait if si else []):
            pi = None
            if w.ant_name and TICK.match(w.ant_name):
                pi = producer(w.ant_name, w.wait_value)
                if pi == i:
                    pi = None
            out.append((w, pi))
        return out

    def compute(idx):
        stack = [idx]
        while stack:
            i = stack[-1]
            if after[i] is not None:
                stack.pop()
                continue
            deps = []
            p = prev_idx[i]
            if p is not None and after[p] is None:
                deps.append(p)
            wps = wait_producers(i)
            for w, pi in wps:
                if pi is not None and after[pi] is None:
                    deps.append(pi)
            if deps:
                stack.extend(deps)
                continue
            stack.pop()
            c = {}
            if p is not None:
                src = before[p] if type(insts[p]).__name__ in ASYNC_T else after[p]
                for s, v in src.items():
                    if c.get(s, -1) < v:
                        c[s] = v
            for w, pi in wps:
                if pi is not None:
                    for s, v in after[pi].items():
                        if c.get(s, -1) < v:
                            c[s] = v
                if w.ant_name and TICK.match(w.ant_name):
                    if c.get(w.ant_name, -1) < w.wait_value:
                        c[w.ant_name] = w.wait_value
            before[i] = c
            a = dict(c)
            for nm, v in own_updates[i]:
                if a.get(nm, -1) < v:
                    a[nm] = v
            after[i] = a

    for i in range(n):
        compute(i)

    # pass 1: transitive reduction of each instruction's wait list
    for i, ins in enumerate(insts):
        si = ins.sync_info
        if si is None or len(si.on_wait) <= 1:
            continue
        if type(ins).__name__ == "InstEventSemaphore":
            continue
        waits = list(si.on_wait)
        p = prev_idx[i]
        base = {}
        if p is not None:
            src = before[p] if type(insts[p]).__name__ in ASYNC_T else after[p]
            base.update(src)
        closures = []
        for w in waits:
            cl = {}
            if w.ant_name and TICK.match(w.ant_name):
                pi = producer(w.ant_name, w.wait_value)
                if pi is not None and pi != i:
                    cl.update(after[pi])
                if cl.get(w.ant_name, -1) < w.wait_value:
                    cl[w.ant_name] = w.wait_value
            closures.append(cl)
        kept = []
        kept_cl = dict(base)
        for j, w in enumerate(waits):
            nm = w.ant_name
            if not (nm and TICK.match(nm)):
                kept.append(w)
                continue
            cov = dict(kept_cl)
            for j2 in range(j + 1, len(waits)):
                for s, v in closures[j2].items():
                    if cov.get(s, -1) < v:
                        cov[s] = v
            if cov.get(nm, -1) >= w.wait_value:
                continue
            kept.append(w)
            for s, v in closures[j].items():
                if kept_cl.get(s, -1) < v:
                    kept_cl[s] = v
        if len(kept) != len(waits):
            si.on_wait = kept
            ins.sync_info = si

    # pass 2: split any instruction still carrying > 1 wait into a chain of
    # same-engine presync drains (each drain fits a single sync command)
    for bb in nc.m.functions[0].blocks:
        while True:
            insns = list(bb.instructions)
            target = None
            for idx, ins in enumerate(insns):
                si = ins.sync_info
                if si is None:
                    continue
                if len(si.on_wait) > 1:
                    target = (idx, ins)
                    break
            if target is None:
                break
            idx, ins = target
            si = ins.sync_info
            waits = list(si.on_wait)
            if type(ins).__name__ == "InstDrain":
                room = max(0, 1 - len(si.on_update))
            else:
                room = 1
            keep, extra = waits[len(waits) - room:], waits[: len(waits) - room]
            pos = idx
            for i, w in enumerate(extra):
                nd = mybir.InstDrain(
                    name=f"{ins.name}-presync{i}", ins=[], outs=[],
                    bass_is_fusable=False,
                )
                nd.engine = ins.engine
                nd.sync_info = _br.SyncInfo(on_wait=[w], on_update=[])
                nc.register_instruction(nd, overwrite=True)
                bb.instructions.insert(pos, nd)
                pos += 1
            si.on_wait = keep
            ins.sync_info = si


def _build():
    nc = bass.Bass(trn_type="TRN2")
    x = nc.dram_tensor("x", [P, SPP * C], F16, kind="ExternalInput")
    g = nc.dram_tensor("g", [P, SPP], F16, kind="ExternalInput")
    cst = nc.dram_tensor("cst", [P, NCONST], F32, kind="ExternalInput")
    st = nc.dram_tensor("st", [P, NCHUNK * NSLOT], F32, kind="ExternalOutput")

    X = x[:, :].rearrange("p (k c) -> p k c", c=C)  # [125, 1000, 100]

    with tile.TileContext(nc) as tc:
        with (
            tc.tile_pool(name="xin", bufs=1) as xin,
            tc.tile_pool(name="persist", bufs=1) as persist,
        ):
            # per-chunk persistent buffers (separate tiles so later-tile
            # writes never alias earlier chunks' binning reads)
            m_ch = [persist.tile([P, hi - lo], F32, tag=f"m{i}", name=f"m{i}")
                    for i, (lo, hi) in enumerate(CHUNKS)]
            s_ch = [persist.tile([P, hi - lo], F32, tag=f"s{i}", name=f"s{i}")
                    for i, (lo, hi) in enumerate(CHUNKS)]
            eg = persist.tile([P, SPP], F16)
            eg32 = persist.tile([P, SPP], F32)
            # separate scratch outputs per engine: a shared one creates
            # cross-engine WAW chains that serialize the whole pipeline
            dump_a = persist.tile([P, 600], F32)
            dump_d = persist.tile([P, 600], F32)
            stats = persist.tile([P, NCHUNK * NSLOT], F32)
            cst_sb = persist.tile([P, NCONST], F32)

            nc.scalar.dma_start(out=eg[:, :], in_=g[:, :])
            nc.scalar.dma_start(out=cst_sb[:, :], in_=cst[:, :])

            # binning ops, built per chunk when it closes and interleaved
            # into later tiles' ACT / DVE streams so they overlap the loop
            act_q = []          # pending ACT binning thunks
            dve_q = []          # pending DVE binning thunks

            def make_binning(cidx):
                lo, hi = CHUNKS[cidx]
                L = hi - lo
                conf = m_ch[cidx][:, :L]
                v = s_ch[cidx][:, :L]
                dmp = dump_a[:, 0:L]
                dmp_d = dump_d[:, 0:L]
                base = cidx * NSLOT

                def wt_op(j):
                    bias = cst_sb[:, j:j + 1] if j else cst_sb[:, 0:1]
                    return lambda: nc.scalar.activation(
                        dmp, conf, ACTF.Relu, bias=bias,
                        accum_out=stats[:, base + j:base + j + 1],
                    )

                def av_op(j):
                    bias = cst_sb[:, 15 + j:16 + j] if j < 14 else cst_sb[:, 29:30]
                    return lambda: nc.scalar.activation(
                        dmp, v, ACTF.Sign, bias=bias,
                        accum_out=stats[:, base + 29 + j:base + 30 + j],
                    )

                def nn_op(j):
                    return lambda: nc.vector.tensor_scalar(
                        dmp_d, conf, float(THR[j]), None,
                        op0=ALU.is_le, op1=ALU.add,
                        accum_out=stats[:, base + 15 + j:base + 16 + j],
                    )

                # DVE direct-count variants for the small tail chunks:
                # slot0 = sum min(conf, 2) = T;  1+b = sum min(conf, C_b);
                # 15+b = #(conf <= C_b);  29+b = #(v' >= -C_b);  43 = #acc
                def wtmin_op(j):
                    cb = 2.0 if j == 0 else float(THR[j - 1])
                    return lambda: nc.vector.tensor_scalar(
                        dmp_d, conf, cb, None, op0=ALU.min, op1=ALU.add,
                        accum_out=stats[:, base + j:base + j + 1],
                    )

                def avge_op(j):
                    tb = -float(THR[j]) if j < 14 else -1.0
                    return lambda: nc.vector.tensor_scalar(
                        dmp_d, v, tb, None, op0=ALU.is_ge, op1=ALU.add,
                        accum_out=stats[:, base + 29 + j:base + 30 + j],
                    )

                def nn_sign_op(j):
                    return lambda: nc.scalar.activation(
                        dmp, conf, ACTF.Sign, bias=cst_sb[:, 1 + j:2 + j],
                        accum_out=stats[:, base + 15 + j:base + 16 + j],
                    )

                if cidx in ACT_CHUNKS:
                    for j in range(15):
                        act_q.append(wt_op(j))
                        act_q.append(av_op(j))
                    for j in range(14):
                        act_q.append(nn_sign_op(j))
                else:
                    for j in range(15):
                        dve_q.append(wtmin_op(j))
                        dve_q.append(avge_op(j))
                    for j in range(14):
                        dve_q.append(nn_op(j))

            def drain_queues(nact, ndve):
                for _ in range(min(nact, len(act_q))):
                    act_q.pop(0)()
                for _ in range(min(ndve, len(dve_q))):
                    dve_q.pop(0)()

            off = 0
            ci = 0
            pending_sum = None     # deferred 25-wide row sum (prev tile)
            for t, k in enumerate(SIZES):
                lo, hi = CHUNKS[ci]
                sl = slice(off - lo, off - lo + k)
                off += k
                m_c, s_c = m_ch[ci], s_ch[ci]

                # explicit per-slot tags: Tile's free-pool reuse is LIFO,
                # which collapses the 4 buffers to ~2 and serializes the
                # pipeline; manual round-robin enforces reuse distance 4
                xt = xin.tile([P, 100, C], F16, tag=f"xt{t % 8}", name=f"xt{t}")
                nc.sync.dma_start(out=xt[:, :k, :], in_=X[:, off - k:off, :])
                nc.scalar.activation(xt[:, :k, :], xt[:, :k, :], ACTF.Exp)
                if t == 2:
                    nc.scalar.activation(eg[:, :], eg[:, :], ACTF.Exp)
                drain_queues(12, 6)
                if k <= DVE_FULL_K:
                    nc.vector.reduce_max(
                        out=m_c[:, sl], in_=xt[:, :k, :], axis=AX.X
                    )
                    nc.vector.reduce_sum(
                        out=s_c[:, sl], in_=xt[:, :k, :], axis=AX.X
                    )
                else:
                    nc.vector.reduce_max(
                        out=m_c[:, sl], in_=xt[:, :k, :], axis=AX.X
                    )
                    # pairwise ADD tree on GpSimd (Pool TT supports add, not
                    # max), in place after DVE's rowmax read.  Level 3 sums
                    # 0:12 += 13:25 leaving element 12 untouched, so the
                    # 13-wide DVE reduce still covers all 25 partials.  The
                    # final DVE row sum is DEFERRED one tile so DVE is not
                    # head-of-line blocked on the Pool engine.
                    nc.gpsimd.tensor_tensor(
                        xt[:, :k, 0:50], xt[:, :k, 0:50], xt[:, :k, 50:100],
                        op=ALU.add,
                    )
                    nc.gpsimd.tensor_tensor(
                        xt[:, :k, 0:25], xt[:, :k, 0:25], xt[:, :k, 25:50],
                        op=ALU.add,
                    )
                    nc.gpsimd.tensor_tensor(
                        xt[:, :k, 0:12], xt[:, :k, 0:12], xt[:, :k, 13:25],
                        op=ALU.add,
                    )
                    if pending_sum is not None:
                        pending_sum()
                    pending_sum = (
                        lambda xt=xt, k=k, s_c=s_c, sl=sl:
                        nc.vector.reduce_sum(
                            out=s_c[:, sl], in_=xt[:, :k, 0:13], axis=AX.X
                        )
                    )
                drain_queues(0, 2)

                if t == CHUNK_LAST_TILE[ci]:
                    if pending_sum is not None:
                        pending_sum()
                        pending_sum = None
                    # chunk epilogue: r = 1/S (DVE); d = eg - maxE (DVE,
                    # in place into eg; d <= 0, == 0 iff correct);
                    # sd = sign(d) (ACT, in place); conf = maxE * r (DVE);
                    # v' = 2*sd - conf (DVE)  [acc1: -conf; acc0: -2-conf]
                    L = hi - lo
                    egc = eg32[:, lo:hi]
                    # widen exp(g) fp16 -> f32 (exact) for the accuracy
                    # compare; the max of E widened by reduce_max is the
                    # same fp16 value, so d == 0 iff label hits the argmax
                    nc.scalar.activation(egc, eg[:, lo:hi], ACTF.Copy)
                    nc.vector.reciprocal(s_c[:, :L], s_c[:, :L])
                    # d' = m - eg via stt with out==in0 (the TT
                    # subtract and out==in1 stt forms run ~4x slower);
                    # Sign(scale=-2) then gives sign(eg - m) in {-1, 0}
                    nc.vector.scalar_tensor_tensor(
                        egc, egc, -1.0, m_c[:, :L],
                        op0=ALU.mult, op1=ALU.add,
                    )
                    nc.scalar.activation(egc, egc, ACTF.Sign, scale=-2.0)
                    nc.gpsimd.tensor_tensor(
                        m_c[:, :L], m_c[:, :L], s_c[:, :L], op=ALU.mult
                    )
                    nc.vector.scalar_tensor_tensor(
                        s_c[:, :L], egc, 2.0, m_c[:, :L],
                        op0=ALU.mult, op1=ALU.subtract,
                    )
                    make_binning(ci)
                    ci = min(ci + 1, len(CHUNKS) - 1)

            drain_queues(len(act_q), len(dve_q))
            nc.sync.dma_start(out=st[:, :], in_=stats[:, :])

    _fix_sync(nc)
    return nc


_NC_CACHE = {}


def _get_nc():
    if "nc" not in _NC_CACHE:
        _NC_CACHE["nc"] = _build()
    return _NC_CACHE["nc"]


def kernel(logits, labels):
    global LAST_RESULTS
    logits = np.asarray(logits)
    labels_i = np.asarray(labels).astype(np.int64)
    assert logits.shape == (N, C), logits.shape
    # fp16 halves the HBM traffic (the measured per-core DMA bandwidth with
    # all 8 cores active is ~130 GB/s, which is THE bottleneck); quantizing
    # the logits to fp16 moves the final ECE by only ~3e-4 relative
    logits = np.ascontiguousarray(logits.astype(np.float16))

    # host-side gather of the label logit (1% of input bytes); fp16 so the
    # device-side exp(g) matches the tile exp bit for bit
    gvals = logits[np.arange(N), labels_i]

    in_maps = []
    for c in range(NCORES):
        sl = slice(c * ROWS, (c + 1) * ROWS)
        in_maps.append(
            {
                "x": logits[sl].reshape(P, SPP * C),
                "g": gvals[sl].reshape(P, SPP),
                "cst": np.tile(CVEC, (P, 1)),
            }
        )

    trace = bool(int(os.environ.get("ECE_TRACE", "0")))
    res = run_bass_kernel_spmd(
        _get_nc(), in_maps, core_ids=list(range(NCORES)), trace=trace
    )
    LAST_RESULTS = res

    tot = np.zeros((NCHUNK, NSLOT), np.float64)
    for out in res.results:
        tot += out["st"].astype(np.float64).reshape(P, NCHUNK, NSLOT).sum(axis=0)

    thr64 = np.array([np.float64(t) for t in THR])
    S = np.zeros(16)
    A = np.zeros(16)
    T_all = 0.0
    acc_all = 0.0
    for c, (lo, hi) in enumerate(CHUNKS):
        n_c = 1000.0 * (hi - lo)               # 8 cores x 125 partitions
        sl = tot[c]
        if c in ACT_CHUNKS:
            nle = (n_c - sl[15:29]) / 2.0      # sign-decoded counts
        else:
            nle = sl[15:29]                    # direct is_le counts
        ngt = n_c - nle
        if c in ACT_CHUNKS:
            T_c = sl[0]
            S[0:14] += T_c - sl[1:15] - thr64[0:14] * ngt
            A[0:14] += (n_c + sl[29:43]) / 2.0
            acc_all += (n_c + sl[43]) / 2.0
        else:
            T_c = sl[0]
            S[0:14] += sl[1:15] - thr64[0:14] * ngt
            A[0:14] += sl[29:43]
            acc_all += sl[43]
        T_all += T_c
    S[14] = T_all
    S[15] = T_all
    A[14] = acc_all
    A[15] = acc_all
    conf_sum = np.diff(S, prepend=0.0)
    acc_sum = np.diff(A, prepend=0.0)
    ece = np.abs(conf_sum - acc_sum).sum() / N
    return np.array([ece], dtype=np.float32)


# revision 31
# speedup vs baseline: 1.2152x; 1.2152x over previous
"""ECE loss kernel for Trainium2 (Bass/Tile), data-parallel over 8 NeuronCores.

Math (per sample row of logits[N, C]):
  conf = max softmax(x) = max(E) / sum(E),  E = exp(x)
  acc  = (argmax(x) == label)  via  exp(g) == max(E), g = x[i, label_i]
  ece  = sum_b |conf_sum[b] - acc_sum[b]| / N   over 15 real bins

Per-core device work (125k rows as [125 partitions x 1000 samples x 100 cls]),
balanced across ALL engines (the previous version put everything on DVE):
  - DMA   (sync HWDGE only): 13 tiles, up to 5 MB each
  - ACT   : E = exp(x) in place; later all per-bin statistics via
            activation(Relu/Sign, bias=-C, accum_out=...) which gives a free
            per-partition sum of the activated values
  - DVE   : rowmax(E); rowsum for the small lead tiles; final rowsum over 25
            for the big tiles; recip/eq/mul/stt epilogue per chunk
  - GpSimd: two pairwise-ADD tree levels (100->50->25) in place on each big
            tile, via tensor_tensor(add) -- runs after DVE's rowmax read
            (Pool TT supports add/mult but not max)

Per-bin statistics (accumulated per chunk of samples so they overlap the
main loop instead of forming a serial tail):
  wt'(Cb) = sum relu(conf - Cb)        (ACT, 15 ops: Cb in {0} + C_0..C_13)
  nle_b   = sum (conf <= C_b)          (DVE tensor_scalar accum, 14 ops)
  q'(Tb)  = sum sign(v' - Tb)          (ACT, 15 ops) where
            v' = 2*sign(eg - maxE) - conf  (acc1: -conf; acc0: -2-conf),
            Tb = -C_b for b=0..13 and -1.0 for the total-acc count
Host recovers:
  T = wt'(0);  S_b = T - wt'_b - C_b*(N - nle_b)
  A_b = (N + q'_b)/2   (cumulative acc counts);  diffs give per-bin sums.
C_b is the exact f32 boundary: the largest f32 y with f32(15*y) <= b+1, so
binning matches the reference's ceil(conf*15) up to ~1-sample tie effects
(~1e-6 relative on the final ECE).
"""

import os

import numpy as np

import concourse.bass as bass
import concourse.mybir as mybir
import concourse.tile as tile
from concourse.bass_utils import run_bass_kernel_spmd

F32 = mybir.dt.float32
F16 = mybir.dt.float16
ALU = mybir.AluOpType
AX = mybir.AxisListType
ACTF = mybir.ActivationFunctionType

N = 1_000_000
C = 100
NCORES = 8
ROWS = N // NCORES          # 125000 rows per core
P = 125                     # SBUF partitions used
SPP = ROWS // P             # 1000 samples per partition

# small tiles at both ends: fast pipeline ramp-up AND a short serial tail
SIZES = [12, 13, 25, 50] + [100] * 8 + [50, 25, 13, 12]   # 16 tiles, 1000
HALF_TILE = 8              # after this tile, samples [0:HALF] are final
HALF = 600
DVE_FULL_K = 25            # tiles with k <= this do the row sum on DVE too

LAST_RESULTS = None         # stashed BassKernelResults for test harness


def _bin_thresholds():
    """C_b = largest f32 y such that f32(15*y) <= b+1, for b = 0..14."""
    thr = []
    for b in range(15):
        tgt = np.float32(b + 1)

        def f(v):
            return np.float32(np.float32(15.0) * v)

        y = np.float32((b + 1) / 15.0)
        if f(y) <= tgt:
            while True:
                y2 = np.nextafter(y, np.float32(np.inf))
                if f(y2) <= tgt:
                    y = y2
                else:
                    break
        else:
            while f(y) > tgt:
                y = np.nextafter(y, np.float32(-np.inf))
        thr.append(np.float32(y))
    return thr


THR = _bin_thresholds()                       # 15 values, b = 0..14

# bias constants shipped as a tiny input tensor (the const-AP pool only has
# 0.0/1.0 pre-registered):  [0] = 0.0 (wt base),  [1+b] = -C_b (wt relu),
# [15+b] = +C_b (av sign on v' = 2*sign(d) - conf),  [29] = +1.0 (acc count)
NCONST = 30
CVEC = np.zeros(NCONST, np.float32)
for _b in range(14):
    CVEC[1 + _b] = -THR[_b]
    CVEC[15 + _b] = THR[_b]
CVEC[29] = np.float32(1.0)


def _fix_sync(nc):
    """Instruction encodings only carry 2 sync-command slots (completion
    update takes one), so every instruction should hold <= 1 wait.  Tile's
    sem emission is not transitively minimal, so: (1) drop waits implied
    transitively through other waits / same-engine program order; (2) split
    any leftover multi-wait instruction into a chain of presync drains."""
    import bisect
    import re

    import bass_rust as _br

    TICK = re.compile(r"^(Activation|DVE|PE|Pool|SP|DMAHW\d+|DMASW\d+)_\d+$")
    ASYNC_T = {"InstDMACopy", "InstTriggerDma"}

    insts = []
    for bb in nc.m.functions[0].blocks:
        for ins in bb.instructions:
            insts.append(ins)
    n = len(insts)

    # producer map: tick sem -> sorted cumulative values + producing inst idx
    prod_vals, prod_idx = {}, {}
    own_updates = [[] for _ in range(n)]
    cum = {}
    for idx, ins in enumerate(insts):
        si = ins.sync_info
        if si is None:
            continue
        for u in si.on_update:
            nm = u.ant_name
            if not nm or not TICK.match(nm):
                continue
            if u.update_mode not in ("sem-inc", "sem-add-imm"):
                continue
            v = cum.get(nm, 0) + (u.update_value or 1)
            cum[nm] = v
            prod_vals.setdefault(nm, []).append(v)
            prod_idx.setdefault(nm, []).append(idx)
            own_updates[idx].append((nm, v))

    def producer(nm, val):
        vs = prod_vals.get(nm)
        if not vs:
            return None
        k = bisect.bisect_left(vs, val)
        if k >= len(vs):
            return None
        return prod_idx[nm][k]

    prev_idx = [None] * n
    last = {}
    for idx, ins in enumerate(insts):
        e = str(getattr(ins, "engine", None))
        prev_idx[idx] = last.get(e)
        last[e] = idx

    # before[i]: sem clock guaranteed when inst i issues (incl its waits)
    # after[i]: clock guaranteed when inst i COMPLETES (incl own updates)
    before = [None] * n
    after = [None] * n

    def wait_producers(i):
        si = insts[i].sync_info
        out = []
        for w in (si.on_wait if si else []):
            pi = None
            if w.ant_name and TICK.match(w.ant_name):
                pi = producer(w.ant_name, w.wait_value)
                if pi == i:
                    pi = None
            out.append((w, pi))
        return out

    def compute(idx):
        stack = [idx]
        while stack:
            i = stack[-1]
            if after[i] is not None:
                stack.pop()
                continue
            deps = []
            p = prev_idx[i]
            if p is not None and after[p] is None:
                deps.append(p)
            wps = wait_producers(i)
            for w, pi in wps:
                if pi is not None and after[pi] is None:
                    deps.append(pi)
            if deps:
                stack.extend(deps)
                continue
            stack.pop()
            c = {}
            if p is not None:
                src = before[p] if type(insts[p]).__name__ in ASYNC_T else after[p]
                for s, v in src.items():
                    if c.get(s, -1) < v:
                        c[s] = v
            for w, pi in wps:
                if pi is not None:
                    for s, v in after[pi].items():
                        if c.get(s, -1) < v:
                            c[s] = v
                if w.ant_name and TICK.match(w.ant_name):
                    if c.get(w.ant_name, -1) < w.wait_value:
                        c[w.ant_name] = w.wait_value
            before[i] = c
            a = dict(c)
            for nm, v in own_updates[i]:
                if a.get(nm, -1) < v:
                    a[nm] = v
            after[i] = a

    for i in range(n):
        compute(i)

    # pass 1: transitive reduction of each instruction's wait list
    for i, ins in enumerate(insts):
        si = ins.sync_info
        if si is None or len(si.on_wait) <= 1:
            continue
        if type(ins).__name__ == "InstEventSemaphore":
            continue
        waits = list(si.on_wait)
        p = prev_idx[i]
        base = {}
        if p is not None:
            src = before[p] if type(insts[p]).__name__ in ASYNC_T else after[p]
            base.update(src)
        closures = []
        for w in waits:
            cl = {}
            if w.ant_name and TICK.match(w.ant_name):
                pi = producer(w.ant_name, w.wait_value)
                if pi is not None and pi != i:
                    cl.update(after[pi])
                if cl.get(w.ant_name, -1) < w.wait_value:
                    cl[w.ant_name] = w.wait_value
            closures.append(cl)
        kept = []
        kept_cl = dict(base)
        for j, w in enumerate(waits):
            nm = w.ant_name
            if not (nm and TICK.match(nm)):
                kept.append(w)
                continue
            cov = dict(kept_cl)
            for j2 in range(j + 1, len(waits)):
                for s, v in closures[j2].items():
                    if cov.get(s, -1) < v:
                        cov[s] = v
            if cov.get(nm, -1) >= w.wait_value:
                continue
            kept.append(w)
            for s, v in closures[j].items():
                if kept_cl.get(s, -1) < v:
                    kept_cl[s] = v
        if len(kept) != len(waits):
            si.on_wait = kept
            ins.sync_info = si

    # pass 2: split any instruction still carrying > 1 wait into a chain of
    # same-engine presync drains (each drain fits a single sync command)
    for bb in nc.m.functions[0].blocks:
        while True:
            insns = list(bb.instructions)
            target = None
            for idx, ins in enumerate(insns):
                si = ins.sync_info
                if si is None:
                    continue
                if len(si.on_wait) > 1:
                    target = (idx, ins)
                    break
            if target is None:
                break
            idx, ins = target
            si = ins.sync_info
            waits = list(si.on_wait)
            if type(ins).__name__ == "InstDrain":
                room = max(0, 1 - len(si.on_update))
            else:
                room = 1
            keep, extra = waits[len(waits) - room:], waits[: len(waits) - room]
            pos = idx
            for i, w in enumerate(extra):
                nd = mybir.InstDrain(
                    name=f"{ins.name}-presync{i}", ins=[], outs=[],
                    bass_is_fusable=False,
                )
                nd.engine = ins.engine
                nd.sync_info = _br.SyncInfo(on_wait=[w], on_update=[])
                nc.register_instruction(nd, overwrite=True)
                bb.instructions.insert(pos, nd)
                pos += 1
            si.on_wait = keep
            ins.sync_info = si


def _build():
    nc = bass.Bass(trn_type="TRN2")
    x = nc.dram_tensor("x", [P, SPP * C], F16, kind="ExternalInput")
    g = nc.dram_tensor("g", [P, SPP], F16, kind="ExternalInput")
    mo = nc.dram_tensor("mo", [P, SPP], F32, kind="ExternalOutput")
    so = nc.dram_tensor("so", [P, SPP], F32, kind="ExternalOutput")
    ego = nc.dram_tensor("ego", [P, SPP], F16, kind="ExternalOutput")

    X = x[:, :].rearrange("p (k c) -> p k c", c=C)  # [125, 1000, 100]

    with tile.TileContext(nc) as tc:
        with (
            tc.tile_pool(name="xin", bufs=1) as xin,
            tc.tile_pool(name="persist", bufs=1) as persist,
        ):
            m_all = persist.tile([P, SPP], F32)
            s_all = persist.tile([P, SPP], F32)
            eg = persist.tile([P, SPP], F16)

            nc.scalar.dma_start(out=eg[:, :], in_=g[:, :])

            off = 0
            pending_sum = None     # deferred 13-wide row sum (prev tile)
            for t, k in enumerate(SIZES):
                sl = slice(off, off + k)
                off += k

                # explicit per-slot tags: Tile's free-pool reuse is LIFO,
                # which collapses the buffers and serializes the pipeline;
                # manual round-robin enforces reuse distance 8
                xt = xin.tile([P, 100, C], F16, tag=f"xt{t % 8}", name=f"xt{t}")
                nc.sync.dma_start(out=xt[:, :k, :], in_=X[:, sl, :])
                nc.scalar.activation(xt[:, :k, :], xt[:, :k, :], ACTF.Exp)
                if t == 2:
                    nc.scalar.activation(eg[:, :], eg[:, :], ACTF.Exp)
                nc.vector.reduce_max(out=m_all[:, sl], in_=xt[:, :k, :], axis=AX.X)
                if k <= DVE_FULL_K:
                    nc.vector.reduce_sum(
                        out=s_all[:, sl], in_=xt[:, :k, :], axis=AX.X
                    )
                else:
                    # pairwise ADD tree on GpSimd (Pool TT supports add, not
                    # max), in place after DVE's rowmax read; the final
                    # 25-wide DVE row sum is DEFERRED one tile so DVE is not
                    # head-of-line blocked on the Pool engine
                    nc.gpsimd.tensor_tensor(
                        xt[:, :k, 0:50], xt[:, :k, 0:50], xt[:, :k, 50:100],
                        op=ALU.add,
                    )
                    nc.gpsimd.tensor_tensor(
                        xt[:, :k, 0:25], xt[:, :k, 0:25], xt[:, :k, 25:50],
                        op=ALU.add,
                    )
                    if pending_sum is not None:
                        pending_sum()
                    pending_sum = (
                        lambda xt=xt, k=k, sl=sl:
                        nc.vector.reduce_sum(
                            out=s_all[:, sl], in_=xt[:, :k, 0:25], axis=AX.X
                        )
                    )
                if t == HALF_TILE:
                    # first-half results are final: ship them now so the
                    # output transfer overlaps the remaining input stream
                    if pending_sum is not None:
                        pending_sum()
                        pending_sum = None
                    nc.scalar.dma_start(out=mo[:, 0:HALF], in_=m_all[:, 0:HALF])
                    nc.scalar.dma_start(out=so[:, 0:HALF], in_=s_all[:, 0:HALF])
                    nc.scalar.dma_start(out=ego[:, :], in_=eg[:, :])

            if pending_sum is not None:
                pending_sum()
            nc.scalar.dma_start(out=mo[:, HALF:], in_=m_all[:, HALF:])
            nc.scalar.dma_start(out=so[:, HALF:], in_=s_all[:, HALF:])

    _fix_sync(nc)
    return nc


_NC_CACHE = {}


def _get_nc():
    if "nc" not in _NC_CACHE:
        _NC_CACHE["nc"] = _build()
    return _NC_CACHE["nc"]


def kernel(logits, labels):
    global LAST_RESULTS
    logits = np.asarray(logits)
    labels_i = np.asarray(labels).astype(np.int64)
    assert logits.shape == (N, C), logits.shape
    # fp16 halves the HBM traffic (the measured per-core DMA bandwidth with
    # all 8 cores active is ~130 GB/s, which is THE bottleneck); quantizing
    # the logits to fp16 moves the final ECE by only ~3e-4 relative
    logits = np.ascontiguousarray(logits.astype(np.float16))

    # host-side gather of the label logit (1% of input bytes); fp16 so the
    # device-side exp(g) matches the tile exp bit for bit
    gvals = logits[np.arange(N), labels_i]

    in_maps = []
    for c in range(NCORES):
        sl = slice(c * ROWS, (c + 1) * ROWS)
        in_maps.append(
            {
                "x": logits[sl].reshape(P, SPP * C),
                "g": gvals[sl].reshape(P, SPP),
            }
        )

    trace = bool(int(os.environ.get("ECE_TRACE", "0")))
    res = run_bass_kernel_spmd(
        _get_nc(), in_maps, core_ids=list(range(NCORES)), trace=trace
    )
    LAST_RESULTS = res

    # device returns per-sample (max E, sum E, exp(g)); the final 16-bin
    # histogram over 1M scalars is trivial host work
    m = np.concatenate([out["mo"].reshape(-1) for out in res.results])
    s = np.concatenate([out["so"].reshape(-1) for out in res.results])
    eg = np.concatenate([out["ego"].reshape(-1) for out in res.results])

    conf = m.astype(np.float64) / s.astype(np.float64)
    acc = (eg.astype(np.float32) == m).astype(np.float64)
    bin_ids = np.clip(np.ceil(conf * 15).astype(np.int64) - 1, 0, 15)
    cs = np.zeros(16)
    as_ = np.zeros(16)
    np.add.at(cs, bin_ids, conf)
    np.add.at(as_, bin_ids, acc)
    ece = np.abs(cs - as_).sum() / N
    return np.array([ece], dtype=np.float32)
